# revision 29
# baseline (speedup 1.0000x reference)
"""Trainium2 Bass kernel for causal self-attention with RoPE.

Problem: B=1, S=2048, D=4096, H=32 heads, head_dim=128, fp32.
Sharding: tensor-parallel over heads across 8 NeuronCores — each core owns 4
heads (column-shard of Wq/Wk/Wv, row-shard of Wo) and produces a partial
[S, D] output; the host sums the 8 partials (the "all-reduce").

Per-core pipeline:
  A: Q^T, K^T = W_h @ hidden^T per head (bf16 inputs, fp32 PSUM), RoPE
     applied during PSUM drain in fp32; K pre-scaled by 1/sqrt(hd).
  B: V = hidden @ Wv^T untransposed (bf16 inputs, fp32 out).
  C: per (head, q-chunk of 512): scores^T = K^T.T @ Q^T chunk in fp32r
     (full-rate PE), causal block-skip, diagonal mask add, exp without
     max-subtraction (scores are O(6); exp is safe in fp32), denominator
     via ones-matmul, out^T = V.T @ exp accumulated in PSUM, normalized by
     1/denom (ones outer-product broadcast).
  D: partial_out = out^T.T @ Wo-shard (fp32r), summed over local heads in
     PSUM, copied to SBUF, DMA'd to DRAM.
"""
import math
import sys

import numpy as np

sys.path.insert(0, "/opt/trn_rl_repo")

import ml_dtypes

import concourse.bass as bass
import concourse.tile as tile
from concourse import bacc
from concourse import mybir
from concourse.bass_utils import run_bass_kernel_spmd

F32 = mybir.dt.float32
F32R = mybir.dt.float32r
BF16 = mybir.dt.bfloat16
EXP = mybir.ActivationFunctionType.Exp

S, D = 2048, 4096
HL = 4            # local heads per core
HD = 128
NJ, CH = 4, 512   # q-chunks
NK = 32           # d-tiles of 128 (contraction for projections)
NCORES = 8


def _r(ap):
    return ap.bitcast(F32R)


def build_nc(reps=1):
    nc = bacc.Bacc("TRN2", target_bir_lowering=False, debug=False,
                   num_devices=NCORES)

    aps = {}
    for nm, shape, dt in (
            ("hT", [D, S], BF16), ("wqT", [D, 512], BF16),
            ("wkT", [D, 512], BF16), ("wvT", [D, 512], BF16),
            ("woT", [512, D], F32),
            ("cosT", [HD, S], F32), ("sinTm", [HD, S], F32),
            ("maskT", [4, HD, CH], F32)):
        aps[nm] = nc.dram_tensor(nm, shape, dt, kind="ExternalInput").ap()
    out = nc.dram_tensor("out", [S, D], BF16, kind="ExternalOutput").ap()

    with tile.TileContext(nc) as tc, \
         nc.allow_low_precision(reason="fp32r tiles for full-rate PE; "
                                "accumulation stays fp32 in PSUM"):
        for _ in range(reps):
            build_body(tc, aps, out)
    nc.compile()
    return nc


def build_body(tc, aps, out):
    nc = tc.nc
    hT = aps["hT"]

    small = tc.alloc_tile_pool(name="small", bufs=1)
    mk = [small.tile([HD, CH], F32, tag=f"mk{t}", name=f"mk{t}")
          for t in range(4)]
    for t in range(4):
        nc.sync.dma_start(mk[t][:], aps["maskT"][t])
    ones_f = small.tile([128, 1], F32, tag="ones_f", name="ones_f")
    nc.vector.memset(ones_f[:], 1.0)
    ones = small.tile([128, 1], F32R, tag="ones", name="ones")
    nc.scalar.copy(ones[:], ones_f[:])
    ones_row_f = small.tile([1, 128], F32, tag="ones_row_f",
                            name="ones_row_f")
    nc.vector.memset(ones_row_f[:], 1.0)
    ones_row = small.tile([1, 128], F32R, tag="ones_row", name="ones_row")
    nc.scalar.copy(ones_row[:], ones_row_f[:])
    wv_sb = small.tile([128, NK * 512], BF16, tag="wv", name="wv_sb")
    for kc in range(4):
        nc.sync.dma_start(
            wv_sb[:, kc*8*512:(kc+1)*8*512].rearrange(
                "p (k f) -> p k f", k=8),
            aps["wvT"][kc*8*128:(kc+1)*8*128, :].rearrange(
                "(k p) f -> p k f", p=128))

    # long-lived tensors on the right SBUF stack; LIFO: pv (after C)
    # pops before pot (after D), so pot is allocated first.
    pot = tc.alloc_tile_pool(name="pot", bufs=1, side="right")
    OT = [pot.tile([HD, S], F32R, tag=f"oT{h}", name=f"oT{h}")
          for h in range(HL)]

    pqk = tc.alloc_tile_pool(name="pqk", bufs=1)
    qT = [pqk.tile([HD, S], BF16, tag=f"qT{h}", name=f"qT{h}")
          for h in range(HL)]
    kTt = [pqk.tile([HD, S], BF16, tag=f"kT{h}", name=f"kT{h}")
           for h in range(HL)]

    # ------------- phase B first: V untransposed (cheap warmup) -------
    # then phase A: Q^T/K^T + RoPE, streamed as head-pair x chunk-pair
    # quarters so each hidden half-tile feeds 8 matmuls.
    pv_pool = tc.alloc_tile_pool(name="pv_pool", bufs=1, side="right")
    V = [pv_pool.tile([128, 512], F32R, tag=f"v{i}", name=f"v{i}")
         for i in range(16)]
    with tc.tile_pool(name="trigA", bufs=1) as trigA, \
         tc.tile_pool(name="wA", bufs=4) as wA, \
         tc.tile_pool(name="htS", bufs=3) as htS, \
         tc.tile_pool(name="ropeA", bufs=2) as ropeA:
        with tc.tile_pool(name="psB", bufs=2, space="PSUM") as psB:
            # prefetch A-phase constants while B computes
            trig = {}
            for nm, srcn in (("cq", "cosT"), ("sq", "sinTm")):
                t = trigA.tile([HD, S], F32, tag=nm, name=f"trig_{nm}")
                nc.sync.dma_start(t[:], aps[srcn])
                trig[nm] = t
            wqk = {}
            for hp in range(2):
                for kg in range(NK // 8):
                    for (tg, srcn) in (("wq", "wqT"), ("wk", "wkT")):
                        wt = wA.tile([128, 8 * 256], BF16, tag=tg,
                                     name=f"{tg}{hp}_{kg}")
                        nc.sync.dma_start(
                            wt.rearrange("p (k f) -> p k f", k=8),
                            aps[srcn][kg*1024:(kg+1)*1024,
                                      hp*256:(hp+1)*256].rearrange(
                                "(k p) f -> p k f", p=128))
                        wqk[(tg, hp, kg)] = wt
            for j in range(NJ):
                pv = [psB.tile([128, 512], F32, tag=f"pv{i}",
                               name=f"pv{j}_{i}") for i in range(4)]
                for k in range(NK):
                    ht = htS.tile([128, CH], BF16, tag="htb",
                                  name=f"htB{j}{k}")
                    nc.sync.dma_start(ht[:],
                                      hT[k*128:(k+1)*128, j*CH:(j+1)*CH])
                    for i in range(4):
                        nc.tensor.matmul(
                            pv[i][:], ht[:, i*128:(i+1)*128],
                            wv_sb[:, k*512:(k+1)*512],
                            start=(k == 0), stop=(k == NK - 1))
                for i in range(4):
                    nc.scalar.copy(V[j*4 + i][:], pv[i][:])

        with tc.tile_pool(name="psA", bufs=1, space="PSUM") as psA:
            for hp in range(2):
                for jp in range(2):
                    pq, pk = {}, {}
                    for hh in range(2):
                        for jj in range(2):
                            pq[(hh, jj)] = psA.tile(
                                [128, CH], F32, tag=f"pq{hh}{jj}",
                                name=f"pq{hp}{jp}{hh}{jj}")
                            pk[(hh, jj)] = psA.tile(
                                [128, CH], F32, tag=f"pk{hh}{jj}",
                                name=f"pk{hp}{jp}{hh}{jj}")
                    for k in range(NK):
                        htf = htS.tile([128, 2 * CH], BF16, tag="htf",
                                       name=f"htA{hp}{jp}{k}")
                        nc.sync.dma_start(
                            htf[:],
                            hT[k*128:(k+1)*128, jp*1024:(jp+1)*1024])
                        for hh in range(2):
                            ko = (k % 8) * 256 + hh * 128
                            wq_t = wqk[("wq", hp, k // 8)][:, ko:ko+128]
                            wk_t = wqk[("wk", hp, k // 8)][:, ko:ko+128]
                            for jj in range(2):
                                rhs = htf[:, jj*CH:(jj+1)*CH]
                                nc.tensor.matmul(
                                    pq[(hh, jj)][:], wq_t, rhs,
                                    start=(k == 0), stop=(k == NK - 1))
                                nc.tensor.matmul(
                                    pk[(hh, jj)][:], wk_t, rhs,
                                    start=(k == 0), stop=(k == NK - 1))
                    # RoPE drains for this quarter
                    for hh in range(2):
                        for jj in range(2):
                            h = hp*2 + hh
                            j = jp*2 + jj
                            for (ps, dst) in ((pq[(hh, jj)], qT[h]),
                                              (pk[(hh, jj)], kTt[h])):
                                cj = trig["cq"][:, j*CH:(j+1)*CH]
                                sj = trig["sq"][:, j*CH:(j+1)*CH]
                                tcos = ropeA.tile([128, CH], F32,
                                                  tag="tcos",
                                                  name=f"tcos{h}{j}")
                                nc.vector.tensor_mul(tcos[:], ps[:], cj)
                                tsin = ropeA.tile([128, CH], F32,
                                                  tag="tsin",
                                                  name=f"tsin{h}{j}")
                                nc.vector.tensor_mul(
                                    tsin[0:64, :], ps[64:128, :],
                                    sj[0:64, :])
                                nc.vector.tensor_mul(
                                    tsin[64:128, :], ps[0:64, :],
                                    sj[64:128, :])
                                nc.vector.tensor_add(
                                    dst[:, j*CH:(j+1)*CH], tcos[:],
                                    tsin[:])

    # ---------------- phase C: attention --------------------------------
    with tc.tile_pool(name="expC", bufs=4) as expC, \
         tc.tile_pool(name="miscC", bufs=2) as miscC, \
         tc.tile_pool(name="psS", bufs=2, space="PSUM") as psS, \
         tc.tile_pool(name="psO", bufs=3, space="PSUM") as psO, \
         tc.tile_pool(name="psD", bufs=2, space="PSUM") as psD:
        for h in range(HL):
            for j in range(NJ):
                nk = 4 * (j + 1)
                po = psO.tile([128, CH], F32, tag="po", name=f"po{h}_{j}")
                pd = psD.tile([1, CH], F32, tag="pd", name=f"pd{h}_{j}")
                qslice = qT[h][:, j*CH:(j+1)*CH]
                for t in range(nk):
                    ps = psS.tile([128, CH], F32, tag="ps",
                                  name=f"ps{h}_{j}_{t}")
                    nc.tensor.matmul(
                        ps[:], kTt[h][:, t*128:(t+1)*128], qslice,
                        start=True, stop=True)
                    if t >= 4 * j:
                        nc.vector.tensor_add(ps[:], ps[:], mk[t - 4*j][:])
                    ex = expC.tile([128, CH], F32R, tag="ex",
                                   name=f"ex{h}{j}{t}")
                    nc.scalar.activation(ex[:], ps[:], EXP)
                    nc.tensor.matmul(
                        po[:], V[t][:, h*128:(h+1)*128], ex[:],
                        start=(t == 0), stop=(t == nk - 1))
                    nc.tensor.matmul(
                        pd[:], ones[:], ex[:],
                        start=(t == 0), stop=(t == nk - 1))
                rec = miscC.tile([1, CH], F32R, tag="rec", name=f"rec{h}{j}")
                nc.vector.reciprocal(rec[:], pd[:])
                rbp = psS.tile([128, CH], F32, tag="ps", name=f"rbp{h}{j}")
                nc.tensor.matmul(rbp[:], ones_row[:], rec[:],
                                 start=True, stop=True)
                rb = miscC.tile([128, CH], F32, tag="rb", name=f"rb{h}{j}")
                nc.scalar.copy(rb[:], rbp[:])
                nc.vector.tensor_mul(OT[h][:, j*CH:(j+1)*CH], po[:], rb[:])
    pqk.release()
    pv_pool.release()

    # ---------------- phase D: output projection ------------------------
    with tc.tile_pool(name="wD", bufs=1) as wD, \
         tc.tile_pool(name="outD", bufs=4) as outD, \
         tc.tile_pool(name="psF", bufs=6, space="PSUM") as psF:
        wo_sb = wD.tile([128, HL * D], F32R, tag="wo", name="wo_sb")
        for h in range(HL):
            for n in range(8):
                ws = outD.tile([128, 512], F32, tag="ws", name=f"ws{h}_{n}")
                nc.sync.dma_start(
                    ws[:], aps["woT"][h*128:(h+1)*128, n*512:(n+1)*512])
                dst = wo_sb[:, h*D + n*512: h*D + (n+1)*512]
                if (h * 8 + n) % 2 == 0:
                    nc.vector.tensor_copy(dst, ws[:])
                else:
                    nc.scalar.copy(dst, ws[:])
        for m in range(16):
            for n in range(8):
                pf = psF.tile([128, 512], F32, tag="pf", name=f"pf{m}_{n}")
                for h in range(HL):
                    nc.tensor.matmul(
                        pf[:], OT[h][:, m*128:(m+1)*128],
                        wo_sb[:, h*D + n*512: h*D + (n+1)*512],
                        start=(h == 0), stop=(h == HL - 1))
                ob = outD.tile([128, 512], BF16, tag="ob", name=f"ob{m}_{n}")
                if (m * 8 + n) % 2 == 0:
                    nc.vector.tensor_copy(ob[:], pf[:])
                else:
                    nc.scalar.copy(ob[:], pf[:])
                nc.sync.dma_start(out[m*128:(m+1)*128, n*512:(n+1)*512],
                                  ob[:])
    pot.release()
    small.release()


def prep_in_maps(hidden_states, attention_mask, cos, sin, Wq, Wk, Wv, Wo):
    bf16 = ml_dtypes.bfloat16
    hs = np.ascontiguousarray(np.asarray(hidden_states)[0], dtype=np.float32)
    mask = np.asarray(attention_mask, np.float32)[0, 0]
    cosT = np.ascontiguousarray(np.asarray(cos, np.float32)[0, 0].T)
    sinT = np.ascontiguousarray(np.asarray(sin, np.float32)[0, 0].T)
    sinTm = np.concatenate([-sinT[:64], sinT[64:]], 0)
    sc = np.float32(1.0 / math.sqrt(HD))
    hiddenT = np.ascontiguousarray(hs.T.astype(bf16))
    maskTd = np.stack([np.ascontiguousarray(mask[0:CH, t*128:(t+1)*128].T)
                       for t in range(4)], 0)

    shared = dict(hT=hiddenT, cosT=cosT, sinTm=np.ascontiguousarray(sinTm),
                  maskT=np.ascontiguousarray(maskTd))
    in_maps = []
    for c in range(NCORES):
        rows = slice(c * 512, (c + 1) * 512)
        in_maps.append(dict(
            shared,
            wqT=np.ascontiguousarray(
                np.asarray(Wq, np.float32)[rows].T.astype(bf16)),
            wkT=np.ascontiguousarray(
                (np.asarray(Wk, np.float32)[rows].T * sc).astype(bf16)),
            wvT=np.ascontiguousarray(
                np.asarray(Wv, np.float32)[rows].T.astype(bf16)),
            woT=np.ascontiguousarray(np.asarray(Wo, np.float32)[:, rows].T),
        ))
    return in_maps


_NC_CACHE = {}


def get_nc():
    if "nc" not in _NC_CACHE:
        _NC_CACHE["nc"] = build_nc()
    return _NC_CACHE["nc"]


def kernel(hidden_states, attention_mask, cos, sin, Wq, Wk, Wv, Wo,
           **run_kwargs):
    in_maps = prep_in_maps(hidden_states, attention_mask, cos, sin,
                           Wq, Wk, Wv, Wo)
    nc = get_nc()
    res = run_bass_kernel_spmd(nc, in_maps, core_ids=list(range(NCORES)),
                               **run_kwargs)
    total = np.zeros((S, D), dtype=np.float32)
    for r in res.results:
        total += np.asarray(r["out"], dtype=np.float32)
    out = total[None]  # [1, S, D]
    _NC_CACHE["last_results"] = res
    return out


# revision 31
# speedup vs baseline: 1.1617x; 1.1617x over previous
"""Trainium2 Bass kernel for causal self-attention with RoPE.

Problem: B=1, S=2048, D=4096, H=32 heads, head_dim=128, fp32.
Sharding: tensor-parallel over heads across 8 NeuronCores — each core owns 4
heads (column-shard of Wq/Wk/Wv, row-shard of Wo) and produces a partial
[S, D] output; the host sums the 8 partials (the "all-reduce").

Per-core pipeline:
  A: Q^T, K^T = W_h @ hidden^T per head (bf16 inputs, fp32 PSUM), RoPE
     applied during PSUM drain in fp32; K pre-scaled by 1/sqrt(hd).
  B: V = hidden @ Wv^T untransposed (bf16 inputs, fp32 out).
  C: per (head, q-chunk of 512): scores^T = K^T.T @ Q^T chunk in fp32r
     (full-rate PE), causal block-skip, diagonal mask add, exp without
     max-subtraction (scores are O(6); exp is safe in fp32), denominator
     via ones-matmul, out^T = V.T @ exp accumulated in PSUM, normalized by
     1/denom (ones outer-product broadcast).
  D: partial_out = out^T.T @ Wo-shard (fp32r), summed over local heads in
     PSUM, copied to SBUF, DMA'd to DRAM.
"""
import math
import sys

import numpy as np

sys.path.insert(0, "/opt/trn_rl_repo")

import ml_dtypes

import concourse.bass as bass
import concourse.tile as tile
from concourse import bacc
from concourse import mybir
from concourse.bass_utils import run_bass_kernel_spmd

F32 = mybir.dt.float32
F32R = mybir.dt.float32r
BF16 = mybir.dt.bfloat16
EXP = mybir.ActivationFunctionType.Exp

S, D = 2048, 4096
HL = 4            # local heads per core
HD = 128
NJ, CH = 4, 512   # q-chunks
NK = 32           # d-tiles of 128 (contraction for projections)
NCORES = 8


def _r(ap):
    return ap.bitcast(F32R)


def build_nc(reps=1):
    nc = bacc.Bacc("TRN2", target_bir_lowering=False, debug=False,
                   num_devices=NCORES)

    aps = {}
    for nm, shape, dt in (
            ("hT", [D, S], BF16), ("wqT", [D, 512], BF16),
            ("wkT", [D, 512], BF16), ("wvT", [D, 512], BF16),
            ("woT", [512, D], BF16),
            ("cosT", [HD, S], F32), ("sinTm", [HD, S], F32),
            ("maskT", [4, HD, CH], F32)):
        aps[nm] = nc.dram_tensor(nm, shape, dt, kind="ExternalInput").ap()
    out = nc.dram_tensor("out", [S, D], BF16, kind="ExternalOutput").ap()

    with tile.TileContext(nc) as tc, \
         nc.allow_low_precision(reason="fp32r tiles for full-rate PE; "
                                "accumulation stays fp32 in PSUM"):
        for _ in range(reps):
            build_body(tc, aps, out)
    nc.compile()
    return nc


def build_body(tc, aps, out):
    nc = tc.nc
    hT = aps["hT"]

    small = tc.alloc_tile_pool(name="small", bufs=1)
    ones_f = small.tile([128, 1], F32, tag="ones_f", name="ones_f")
    nc.vector.memset(ones_f[:], 1.0)
    ones = small.tile([128, 1], F32R, tag="ones", name="ones")
    nc.scalar.copy(ones[:], ones_f[:])
    ones_row_f = small.tile([1, 128], F32, tag="ones_row_f",
                            name="ones_row_f")
    nc.vector.memset(ones_row_f[:], 1.0)
    ones_row = small.tile([1, 128], F32R, tag="ones_row", name="ones_row")
    nc.scalar.copy(ones_row[:], ones_row_f[:])
    mk = [small.tile([HD, CH], F32, tag=f"mk{t}", name=f"mk{t}")
          for t in range(4)]
    wv_sb = small.tile([128, NK * 512], BF16, tag="wv", name="wv_sb")
    kc0 = 0
    for nk_c in (2, 2, 4, 8, 8, 8):
        nc.sync.dma_start(
            wv_sb[:, kc0*512:(kc0+nk_c)*512].rearrange(
                "p (k f) -> p k f", k=nk_c),
            aps["wvT"][kc0*128:(kc0+nk_c)*128, :].rearrange(
                "(k p) f -> p k f", p=128))
        kc0 += nk_c
    for t in range(4):
        nc.sync.dma_start(mk[t][:], aps["maskT"][t])

    # long-lived tensors on the right SBUF stack; LIFO: pv (after C)
    # pops before pot (after D), so pot is allocated first.
    pot = tc.alloc_tile_pool(name="pot", bufs=1, side="right")
    OT = [pot.tile([HD, S], BF16, tag=f"oT{h}", name=f"oT{h}")
          for h in range(HL)]

    pqk = tc.alloc_tile_pool(name="pqk", bufs=1)
    qT = [pqk.tile([HD, S], BF16, tag=f"qT{h}", name=f"qT{h}")
          for h in range(HL)]
    kTt = [pqk.tile([HD, S], BF16, tag=f"kT{h}", name=f"kT{h}")
           for h in range(HL)]

    # ------------- phase B first: V untransposed (cheap warmup) -------
    # then phase A: Q^T/K^T + RoPE, streamed as head-pair x chunk-pair
    # quarters so each hidden half-tile feeds 8 matmuls.
    pv_pool = tc.alloc_tile_pool(name="pv_pool", bufs=1, side="right")
    V = [pv_pool.tile([128, 512], F32R, tag=f"v{i}", name=f"v{i}")
         for i in range(16)]
    with tc.tile_pool(name="trigA", bufs=1) as trigA, \
         tc.tile_pool(name="wA", bufs=4) as wA, \
         tc.tile_pool(name="htS", bufs=3) as htS, \
         tc.tile_pool(name="ropeA", bufs=2) as ropeA:
        with tc.tile_pool(name="psB", bufs=2, space="PSUM") as psB:
            # prefetch A-phase constants while B computes
            trig = {}
            for nm, srcn in (("cq", "cosT"), ("sq", "sinTm")):
                t = trigA.tile([HD, S], F32, tag=nm, name=f"trig_{nm}")
                nc.sync.dma_start(t[:], aps[srcn])
                trig[nm] = t
            wqk = {}
            for hp in range(2):
                for kg in range(NK // 8):
                    for (tg, srcn) in (("wq", "wqT"), ("wk", "wkT")):
                        wt = wA.tile([128, 8 * 256], BF16, tag=tg,
                                     name=f"{tg}{hp}_{kg}")
                        nc.sync.dma_start(
                            wt.rearrange("p (k f) -> p k f", k=8),
                            aps[srcn][kg*1024:(kg+1)*1024,
                                      hp*256:(hp+1)*256].rearrange(
                                "(k p) f -> p k f", p=128))
                        wqk[(tg, hp, kg)] = wt
            for j in range(NJ):
                pv = [psB.tile([128, 512], F32, tag=f"pv{i}",
                               name=f"pv{j}_{i}") for i in range(4)]
                for k in range(NK):
                    ht = htS.tile([128, CH], BF16, tag="htb",
                                  name=f"htB{j}{k}")
                    nc.sync.dma_start(ht[:],
                                      hT[k*128:(k+1)*128, j*CH:(j+1)*CH])
                    for i in range(4):
                        nc.tensor.matmul(
                            pv[i][:], ht[:, i*128:(i+1)*128],
                            wv_sb[:, k*512:(k+1)*512],
                            start=(k == 0), stop=(k == NK - 1))
                for i in range(4):
                    nc.scalar.copy(V[j*4 + i][:], pv[i][:])

        with tc.tile_pool(name="psA", bufs=1, space="PSUM") as psA:
            for hp in range(2):
                for jp in range(2):
                    pq, pk = {}, {}
                    for hh in range(2):
                        for jj in range(2):
                            pq[(hh, jj)] = psA.tile(
                                [128, CH], F32, tag=f"pq{hh}{jj}",
                                name=f"pq{hp}{jp}{hh}{jj}")
                            pk[(hh, jj)] = psA.tile(
                                [128, CH], F32, tag=f"pk{hh}{jj}",
                                name=f"pk{hp}{jp}{hh}{jj}")
                    for k in range(NK):
                        htf = htS.tile([128, 2 * CH], BF16, tag="htf",
                                       name=f"htA{hp}{jp}{k}")
                        nc.sync.dma_start(
                            htf[:],
                            hT[k*128:(k+1)*128, jp*1024:(jp+1)*1024])
                        for hh in range(2):
                            ko = (k % 8) * 256 + hh * 128
                            wq_t = wqk[("wq", hp, k // 8)][:, ko:ko+128]
                            wk_t = wqk[("wk", hp, k // 8)][:, ko:ko+128]
                            for jj in range(2):
                                rhs = htf[:, jj*CH:(jj+1)*CH]
                                nc.tensor.matmul(
                                    pq[(hh, jj)][:], wq_t, rhs,
                                    start=(k == 0), stop=(k == NK - 1))
                                nc.tensor.matmul(
                                    pk[(hh, jj)][:], wk_t, rhs,
                                    start=(k == 0), stop=(k == NK - 1))
                    # RoPE drains for this quarter
                    for hh in range(2):
                        for jj in range(2):
                            h = hp*2 + hh
                            j = jp*2 + jj
                            for (ps, dst) in ((pq[(hh, jj)], qT[h]),
                                              (pk[(hh, jj)], kTt[h])):
                                cj = trig["cq"][:, j*CH:(j+1)*CH]
                                sj = trig["sq"][:, j*CH:(j+1)*CH]
                                tcos = ropeA.tile([128, CH], F32,
                                                  tag="tcos",
                                                  name=f"tcos{h}{j}")
                                nc.vector.tensor_mul(tcos[:], ps[:], cj)
                                tsin = ropeA.tile([128, CH], F32,
                                                  tag="tsin",
                                                  name=f"tsin{h}{j}")
                                nc.vector.tensor_mul(
                                    tsin[0:64, :], ps[64:128, :],
                                    sj[0:64, :])
                                nc.vector.tensor_mul(
                                    tsin[64:128, :], ps[0:64, :],
                                    sj[64:128, :])
                                nc.vector.tensor_add(
                                    dst[:, j*CH:(j+1)*CH], tcos[:],
                                    tsin[:])

    # --------- phase C+D merged, chunk-major: attention then the -------
    # --------- output projection for that chunk's 4 m-tiles       -------
    with tc.tile_pool(name="wD", bufs=1) as wD, \
         tc.tile_pool(name="expC", bufs=4) as expC, \
         tc.tile_pool(name="miscC", bufs=2) as miscC, \
         tc.tile_pool(name="outD", bufs=4) as outD, \
         tc.tile_pool(name="psS", bufs=2, space="PSUM") as psS, \
         tc.tile_pool(name="psO", bufs=2, space="PSUM") as psO, \
         tc.tile_pool(name="psD", bufs=1, space="PSUM") as psD, \
         tc.tile_pool(name="psF", bufs=2, space="PSUM") as psF:
        wo_sb = wD.tile([128, HL * D], BF16, tag="wo", name="wo_sb")
        nc.sync.dma_start(
            wo_sb.rearrange("p (h n) -> p h n", h=HL),
            aps["woT"].rearrange("(h p) n -> p h n", p=128))
        for j in range(NJ):
            nk = 4 * (j + 1)
            for h in range(HL):
                po = psO.tile([128, CH], F32, tag="po", name=f"po{h}_{j}")
                pd = psD.tile([1, CH], F32, tag="pd", name=f"pd{h}_{j}")
                qslice = qT[h][:, j*CH:(j+1)*CH]
                for t in range(nk):
                    ps = psS.tile([128, CH], F32, tag="ps",
                                  name=f"ps{h}_{j}_{t}")
                    nc.tensor.matmul(
                        ps[:], kTt[h][:, t*128:(t+1)*128], qslice,
                        start=True, stop=True)
                    if t >= 4 * j:
                        nc.vector.tensor_add(ps[:], ps[:], mk[t - 4*j][:])
                    ex = expC.tile([128, CH], F32R, tag="ex",
                                   name=f"ex{h}{j}{t}")
                    nc.scalar.activation(ex[:], ps[:], EXP)
                    nc.tensor.matmul(
                        po[:], V[t][:, h*128:(h+1)*128], ex[:],
                        start=(t == 0), stop=(t == nk - 1))
                    nc.tensor.matmul(
                        pd[:], ones[:], ex[:],
                        start=(t == 0), stop=(t == nk - 1))
                rec = miscC.tile([1, CH], F32R, tag="rec",
                                 name=f"rec{h}{j}")
                nc.vector.reciprocal(rec[:], pd[:])
                rbp = psS.tile([128, CH], F32, tag="ps", name=f"rbp{h}{j}")
                nc.tensor.matmul(rbp[:], ones_row[:], rec[:],
                                 start=True, stop=True)
                rb = miscC.tile([128, CH], F32, tag="rb", name=f"rb{h}{j}")
                nc.scalar.copy(rb[:], rbp[:])
                nc.vector.tensor_mul(OT[h][:, j*CH:(j+1)*CH], po[:], rb[:])
            # output projection for this chunk's four m-tiles
            for m in range(4*j, 4*j + 4):
                for n in range(8):
                    pf = psF.tile([128, 512], F32, tag="pf",
                                  name=f"pf{m}_{n}")
                    for h in range(HL):
                        nc.tensor.matmul(
                            pf[:], OT[h][:, m*128:(m+1)*128],
                            wo_sb[:, h*D + n*512: h*D + (n+1)*512],
                            start=(h == 0), stop=(h == HL - 1))
                    ob = outD.tile([128, 512], BF16, tag="ob",
                                   name=f"ob{m}_{n}")
                    if (m * 8 + n) % 2 == 0:
                        nc.vector.tensor_copy(ob[:], pf[:])
                    else:
                        nc.scalar.copy(ob[:], pf[:])
                    nc.sync.dma_start(
                        out[m*128:(m+1)*128, n*512:(n+1)*512], ob[:])
    pqk.release()
    pv_pool.release()
    pot.release()
    small.release()


def prep_in_maps(hidden_states, attention_mask, cos, sin, Wq, Wk, Wv, Wo):
    bf16 = ml_dtypes.bfloat16
    hs = np.ascontiguousarray(np.asarray(hidden_states)[0], dtype=np.float32)
    mask = np.asarray(attention_mask, np.float32)[0, 0]
    cosT = np.ascontiguousarray(np.asarray(cos, np.float32)[0, 0].T)
    sinT = np.ascontiguousarray(np.asarray(sin, np.float32)[0, 0].T)
    sinTm = np.concatenate([-sinT[:64], sinT[64:]], 0)
    sc = np.float32(1.0 / math.sqrt(HD))
    hiddenT = np.ascontiguousarray(hs.T.astype(bf16))
    maskTd = np.stack([np.ascontiguousarray(mask[0:CH, t*128:(t+1)*128].T)
                       for t in range(4)], 0)

    shared = dict(hT=hiddenT, cosT=cosT, sinTm=np.ascontiguousarray(sinTm),
                  maskT=np.ascontiguousarray(maskTd))
    in_maps = []
    for c in range(NCORES):
        rows = slice(c * 512, (c + 1) * 512)
        in_maps.append(dict(
            shared,
            wqT=np.ascontiguousarray(
                np.asarray(Wq, np.float32)[rows].T.astype(bf16)),
            wkT=np.ascontiguousarray(
                (np.asarray(Wk, np.float32)[rows].T * sc).astype(bf16)),
            wvT=np.ascontiguousarray(
                np.asarray(Wv, np.float32)[rows].T.astype(bf16)),
            woT=np.ascontiguousarray(
                np.asarray(Wo, np.float32)[:, rows].T.astype(bf16)),
        ))
    return in_maps


_NC_CACHE = {}


def get_nc():
    if "nc" not in _NC_CACHE:
        _NC_CACHE["nc"] = build_nc()
    return _NC_CACHE["nc"]


def kernel(hidden_states, attention_mask, cos, sin, Wq, Wk, Wv, Wo,
           **run_kwargs):
    in_maps = prep_in_maps(hidden_states, attention_mask, cos, sin,
                           Wq, Wk, Wv, Wo)
    nc = get_nc()
    res = run_bass_kernel_spmd(nc, in_maps, core_ids=list(range(NCORES)),
                               **run_kwargs)
    total = np.zeros((S, D), dtype=np.float32)
    for r in res.results:
        total += np.asarray(r["out"], dtype=np.float32)
    out = total[None]  # [1, S, D]
    _NC_CACHE["last_results"] = res
    return out


# revision 35
# speedup vs baseline: 1.6223x; 1.3965x over previous
"""Trainium2 Bass kernel for causal self-attention with RoPE.

Problem: B=1, S=2048, D=4096, H=32 heads, head_dim=128, fp32.
Sharding: tensor-parallel over heads across 8 NeuronCores — each core owns 4
heads (column-shard of Wq/Wk/Wv, row-shard of Wo) and produces a partial
[S, D] output; the host sums the 8 partials (the "all-reduce").

Per-core pipeline:
  A: Q^T, K^T = W_h @ hidden^T per head (bf16 inputs, fp32 PSUM), RoPE
     applied during PSUM drain in fp32; K pre-scaled by 1/sqrt(hd).
  B: V = hidden @ Wv^T untransposed (bf16 inputs, fp32 out).
  C+D merged, chunk-major: per q-chunk of 512, for each head: scores^T =
     K^T.T @ Q^T chunk (bf16 operands, fp32 PSUM), causal block-skip,
     diagonal mask add, exp without max-subtraction (scores are O(6); exp
     is safe in fp32), denominator via ones-matmul, out^T = V.T @ exp
     accumulated in PSUM (fp32r), normalized by 1/denom (ones
     outer-product broadcast); then immediately the output projection for
     that chunk's four m-tiles (bf16 Wo), so projection matmuls fill the
     attention pipeline tails.
"""
import math
import sys

import numpy as np

sys.path.insert(0, "/opt/trn_rl_repo")

import ml_dtypes

import concourse.bass as bass
import concourse.tile as tile
from concourse import bacc
from concourse import mybir
from concourse.bass_utils import run_bass_kernel_spmd

F32 = mybir.dt.float32
F32R = mybir.dt.float32r
BF16 = mybir.dt.bfloat16
EXP = mybir.ActivationFunctionType.Exp

S, D = 2048, 4096
HL = 4            # local heads per core
HD = 128
NJ, CH = 4, 512   # q-chunks
NK = 32           # d-tiles of 128 (contraction for projections)
NCORES = 8


def _r(ap):
    return ap.bitcast(F32R)


def build_nc(reps=1):
    nc = bacc.Bacc("TRN2", target_bir_lowering=False, debug=False,
                   num_devices=NCORES)

    aps = {}
    for nm, shape, dt in (
            ("hT", [D, S], BF16), ("wqT", [D, 512], BF16),
            ("wkT", [D, 512], BF16), ("wvT", [D, 512], BF16),
            ("woT", [512, D], BF16),
            ("cosT", [HD, S], F32), ("sinTm", [HD, S], F32),
            ("maskT", [4, HD, CH], F32)):
        aps[nm] = nc.dram_tensor(nm, shape, dt, kind="ExternalInput").ap()
    out = nc.dram_tensor("out", [S, D], BF16, kind="ExternalOutput").ap()

    with tile.TileContext(nc) as tc, \
         nc.allow_low_precision(reason="fp32r tiles for full-rate PE; "
                                "accumulation stays fp32 in PSUM"):
        for _ in range(reps):
            build_body(tc, aps, out)
    nc.compile()
    return nc


def build_body(tc, aps, out):
    nc = tc.nc
    hT = aps["hT"]

    small = tc.alloc_tile_pool(name="small", bufs=1)
    ones_f = small.tile([128, 1], F32, tag="ones_f", name="ones_f")
    nc.vector.memset(ones_f[:], 1.0)
    ones = small.tile([128, 1], F32R, tag="ones", name="ones")
    nc.scalar.copy(ones[:], ones_f[:])
    ones_row_f = small.tile([1, 128], F32, tag="ones_row_f",
                            name="ones_row_f")
    nc.vector.memset(ones_row_f[:], 1.0)
    ones_row = small.tile([1, 128], F32R, tag="ones_row", name="ones_row")
    nc.scalar.copy(ones_row[:], ones_row_f[:])
    mk = [small.tile([HD, CH], F32, tag=f"mk{t}", name=f"mk{t}")
          for t in range(4)]
    wv_sb = small.tile([128, NK * 512], BF16, tag="wv", name="wv_sb")
    kc0 = 0
    for nk_c in (2, 2, 4, 8, 8, 8):
        nc.sync.dma_start(
            wv_sb[:, kc0*512:(kc0+nk_c)*512].rearrange(
                "p (k f) -> p k f", k=nk_c),
            aps["wvT"][kc0*128:(kc0+nk_c)*128, :].rearrange(
                "(k p) f -> p k f", p=128))
        kc0 += nk_c

    # long-lived tensors on the right SBUF stack; LIFO: pv (after C)
    # pops before pot (after D), so pot is allocated first.
    pot = tc.alloc_tile_pool(name="pot", bufs=1, side="right")
    OT = [pot.tile([HD, S], BF16, tag=f"oT{h}", name=f"oT{h}")
          for h in range(HL)]

    pqk = tc.alloc_tile_pool(name="pqk", bufs=1)
    qT = [pqk.tile([HD, S], BF16, tag=f"qT{h}", name=f"qT{h}")
          for h in range(HL)]
    kTt = [pqk.tile([HD, S], BF16, tag=f"kT{h}", name=f"kT{h}")
           for h in range(HL)]

    # ------------- phase B first: V untransposed (cheap warmup) -------
    # then phase A: Q^T/K^T + RoPE, streamed as head-pair x chunk-pair
    # quarters so each hidden half-tile feeds 8 matmuls.
    pv_pool = tc.alloc_tile_pool(name="pv_pool", bufs=1, side="right")
    V = [pv_pool.tile([128, 512], F32R, tag=f"v{i}", name=f"v{i}")
         for i in range(16)]
    with tc.tile_pool(name="trigA", bufs=1) as trigA, \
         tc.tile_pool(name="wA", bufs=6) as wA, \
         tc.tile_pool(name="htS", bufs=4) as htS, \
         tc.tile_pool(name="ropeA", bufs=2) as ropeA:
        with tc.tile_pool(name="psB", bufs=2, space="PSUM") as psB:
            # critical-path first: the opening hidden tiles of phase B
            preht = {}
            for k in range(4):
                ht = htS.tile([128, CH], BF16, tag="htb", name=f"htB0{k}")
                nc.sync.dma_start(ht[:], hT[k*128:(k+1)*128, 0:CH])
                preht[k] = ht
            trig = {}
            wqk = {}

            def emit_prefetch():
                # A-phase constants; emitted after B's first chunk so the
                # opening hidden stream isn't queued behind 10 MB of DMA
                for nm, srcn in (("cq", "cosT"), ("sq", "sinTm")):
                    t = trigA.tile([HD, S], F32, tag=nm, name=f"trig_{nm}")
                    nc.sync.dma_start(t[:], aps[srcn])
                    trig[nm] = t
                for hp in range(2):
                    for kg in range(NK // 8):
                        for (tg, srcn) in (("wq", "wqT"), ("wk", "wkT")):
                            wt = wA.tile([128, 8 * 256], BF16, tag=tg,
                                         name=f"{tg}{hp}_{kg}")
                            nc.sync.dma_start(
                                wt.rearrange("p (k f) -> p k f", k=8),
                                aps[srcn][kg*1024:(kg+1)*1024,
                                          hp*256:(hp+1)*256].rearrange(
                                    "(k p) f -> p k f", p=128))
                            wqk[(tg, hp, kg)] = wt

            for j in range(NJ):
                if j == 1:
                    emit_prefetch()
                pv = [psB.tile([128, 512], F32, tag=f"pv{i}",
                               name=f"pv{j}_{i}") for i in range(4)]
                for k in range(NK):
                    if j == 0 and k < 4:
                        ht = preht[k]
                    else:
                        ht = htS.tile([128, CH], BF16, tag="htb",
                                      name=f"htB{j}{k}")
                        nc.sync.dma_start(
                            ht[:], hT[k*128:(k+1)*128, j*CH:(j+1)*CH])
                    for i in range(4):
                        nc.tensor.matmul(
                            pv[i][:], ht[:, i*128:(i+1)*128],
                            wv_sb[:, k*512:(k+1)*512],
                            start=(k == 0), stop=(k == NK - 1))
                for i in range(4):
                    nc.scalar.copy(V[j*4 + i][:], pv[i][:])

        for t in range(4):
            nc.sync.dma_start(mk[t][:], aps["maskT"][t])
        with tc.tile_pool(name="psA", bufs=1, space="PSUM") as psA:
            for hp in range(2):
                for jp in range(2):
                    pq, pk = {}, {}
                    for hh in range(2):
                        for jj in range(2):
                            pq[(hh, jj)] = psA.tile(
                                [128, CH], F32, tag=f"pq{hh}{jj}",
                                name=f"pq{hp}{jp}{hh}{jj}")
                            pk[(hh, jj)] = psA.tile(
                                [128, CH], F32, tag=f"pk{hh}{jj}",
                                name=f"pk{hp}{jp}{hh}{jj}")
                    for k in range(NK):
                        htf = htS.tile([128, 2 * CH], BF16, tag="htf",
                                       name=f"htA{hp}{jp}{k}")
                        nc.sync.dma_start(
                            htf[:],
                            hT[k*128:(k+1)*128, jp*1024:(jp+1)*1024])
                        for hh in range(2):
                            ko = (k % 8) * 256 + hh * 128
                            wq_t = wqk[("wq", hp, k // 8)][:, ko:ko+128]
                            wk_t = wqk[("wk", hp, k // 8)][:, ko:ko+128]
                            for jj in range(2):
                                rhs = htf[:, jj*CH:(jj+1)*CH]
                                nc.tensor.matmul(
                                    pq[(hh, jj)][:], wq_t, rhs,
                                    start=(k == 0), stop=(k == NK - 1))
                                nc.tensor.matmul(
                                    pk[(hh, jj)][:], wk_t, rhs,
                                    start=(k == 0), stop=(k == NK - 1))
                    # RoPE drains for this quarter
                    for hh in range(2):
                        for jj in range(2):
                            h = hp*2 + hh
                            j = jp*2 + jj
                            for (ps, dst) in ((pq[(hh, jj)], qT[h]),
                                              (pk[(hh, jj)], kTt[h])):
                                cj = trig["cq"][:, j*CH:(j+1)*CH]
                                sj = trig["sq"][:, j*CH:(j+1)*CH]
                                tcos = ropeA.tile([128, CH], F32,
                                                  tag="tcos",
                                                  name=f"tcos{h}{j}")
                                nc.vector.tensor_mul(tcos[:], ps[:], cj)
                                tsin = ropeA.tile([128, CH], F32,
                                                  tag="tsin",
                                                  name=f"tsin{h}{j}")
                                nc.vector.tensor_mul(
                                    tsin[0:64, :], ps[64:128, :],
                                    sj[0:64, :])
                                nc.vector.tensor_mul(
                                    tsin[64:128, :], ps[0:64, :],
                                    sj[64:128, :])
                                nc.vector.tensor_add(
                                    dst[:, j*CH:(j+1)*CH], tcos[:],
                                    tsin[:])

    # --------- phase C+D merged, chunk-major: attention then the -------
    # --------- output projection for that chunk's 4 m-tiles       -------
    with tc.tile_pool(name="wD", bufs=1) as wD, \
         tc.tile_pool(name="expC", bufs=4) as expC, \
         tc.tile_pool(name="miscC", bufs=2) as miscC, \
         tc.tile_pool(name="outD", bufs=4) as outD, \
         tc.tile_pool(name="psS", bufs=2, space="PSUM") as psS, \
         tc.tile_pool(name="psO", bufs=2, space="PSUM") as psO, \
         tc.tile_pool(name="psD", bufs=1, space="PSUM") as psD, \
         tc.tile_pool(name="psF", bufs=2, space="PSUM") as psF:
        wo_sb = wD.tile([128, HL * D], BF16, tag="wo", name="wo_sb")
        nc.sync.dma_start(
            wo_sb.rearrange("p (h n) -> p h n", h=HL),
            aps["woT"].rearrange("(h p) n -> p h n", p=128))
        for j in range(NJ):
            nk = 4 * (j + 1)
            for h in range(HL):
                po = psO.tile([128, CH], F32, tag="po", name=f"po{h}_{j}")
                pd = psD.tile([1, CH], F32, tag="pd", name=f"pd{h}_{j}")
                qslice = qT[h][:, j*CH:(j+1)*CH]
                for t in range(nk):
                    ps = psS.tile([128, CH], F32, tag="ps",
                                  name=f"ps{h}_{j}_{t}")
                    nc.tensor.matmul(
                        ps[:], kTt[h][:, t*128:(t+1)*128], qslice,
                        start=True, stop=True)
                    if t >= 4 * j:
                        nc.vector.tensor_add(ps[:], ps[:], mk[t - 4*j][:])
                    ex = expC.tile([128, CH], F32R, tag="ex",
                                   name=f"ex{h}{j}{t}")
                    nc.scalar.activation(ex[:], ps[:], EXP)
                    nc.tensor.matmul(
                        po[:], V[t][:, h*128:(h+1)*128], ex[:],
                        start=(t == 0), stop=(t == nk - 1))
                    nc.tensor.matmul(
                        pd[:], ones[:], ex[:],
                        start=(t == 0), stop=(t == nk - 1))
                rec = miscC.tile([1, CH], F32R, tag="rec",
                                 name=f"rec{h}{j}")
                nc.vector.reciprocal(rec[:], pd[:])
                rbp = psS.tile([128, CH], F32, tag="ps", name=f"rbp{h}{j}")
                nc.tensor.matmul(rbp[:], ones_row[:], rec[:],
                                 start=True, stop=True)
                rb = miscC.tile([128, CH], F32, tag="rb", name=f"rb{h}{j}")
                nc.scalar.copy(rb[:], rbp[:])
                nc.vector.tensor_mul(OT[h][:, j*CH:(j+1)*CH], po[:], rb[:])
            # output projection for this chunk's four m-tiles
            for m in range(4*j, 4*j + 4):
                for n in range(8):
                    pf = psF.tile([128, 512], F32, tag="pf",
                                  name=f"pf{m}_{n}")
                    for h in range(HL):
                        nc.tensor.matmul(
                            pf[:], OT[h][:, m*128:(m+1)*128],
                            wo_sb[:, h*D + n*512: h*D + (n+1)*512],
                            start=(h == 0), stop=(h == HL - 1))
                    ob = outD.tile([128, 512], BF16, tag="ob",
                                   name=f"ob{m}_{n}")
                    if (m * 8 + n) % 2 == 0:
                        nc.vector.tensor_copy(ob[:], pf[:])
                    else:
                        nc.scalar.copy(ob[:], pf[:])
                    nc.sync.dma_start(
                        out[m*128:(m+1)*128, n*512:(n+1)*512], ob[:])
    pqk.release()
    pv_pool.release()
    pot.release()
    small.release()


def prep_in_maps(hidden_states, attention_mask, cos, sin, Wq, Wk, Wv, Wo):
    bf16 = ml_dtypes.bfloat16
    hs = np.ascontiguousarray(np.asarray(hidden_states)[0], dtype=np.float32)
    mask = np.asarray(attention_mask, np.float32)[0, 0]
    cosT = np.ascontiguousarray(np.asarray(cos, np.float32)[0, 0].T)
    sinT = np.ascontiguousarray(np.asarray(sin, np.float32)[0, 0].T)
    sinTm = np.concatenate([-sinT[:64], sinT[64:]], 0)
    sc = np.float32(1.0 / math.sqrt(HD))
    hiddenT = np.ascontiguousarray(hs.T.astype(bf16))
    maskTd = np.stack([np.ascontiguousarray(mask[0:CH, t*128:(t+1)*128].T)
                       for t in range(4)], 0)

    shared = dict(hT=hiddenT, cosT=cosT, sinTm=np.ascontiguousarray(sinTm),
                  maskT=np.ascontiguousarray(maskTd))
    in_maps = []
    for c in range(NCORES):
        rows = slice(c * 512, (c + 1) * 512)
        in_maps.append(dict(
            shared,
            wqT=np.ascontiguousarray(
                np.asarray(Wq, np.float32)[rows].T.astype(bf16)),
            wkT=np.ascontiguousarray(
                (np.asarray(Wk, np.float32)[rows].T * sc).astype(bf16)),
            wvT=np.ascontiguousarray(
                np.asarray(Wv, np.float32)[rows].T.astype(bf16)),
            woT=np.ascontiguousarray(
                np.asarray(Wo, np.float32)[:, rows].T.astype(bf16)),
        ))
    return in_maps


_NC_CACHE = {}


def get_nc():
    if "nc" not in _NC_CACHE:
        _NC_CACHE["nc"] = build_nc()
    return _NC_CACHE["nc"]


def kernel(hidden_states, attention_mask, cos, sin, Wq, Wk, Wv, Wo,
           **run_kwargs):
    in_maps = prep_in_maps(hidden_states, attention_mask, cos, sin,
                           Wq, Wk, Wv, Wo)
    nc = get_nc()
    res = run_bass_kernel_spmd(nc, in_maps, core_ids=list(range(NCORES)),
                               **run_kwargs)
    total = np.zeros((S, D), dtype=np.float32)
    for r in res.results:
        total += np.asarray(r["out"], dtype=np.float32)
    out = total[None]  # [1, S, D]
    _NC_CACHE["last_results"] = res
    return out
